# revision 58
# baseline (speedup 1.0000x reference)
"""2-layer GCN (gnn_message_passing) on 8 Trainium2 NeuronCores.

Source-sharded scatter with split ReduceScatter collectives:
  Edges live on the core owning the SOURCE node (col); dest rows are
  grouped into 896-row cells (7 blocks of 128) over the padded global
  row space. Per core:
    tab1 = fp8e4(X_local @ W1)   (12544 rows, 256B stride, DRAM)
    L1 scatter, per cell: dma_gather table rows by local col idx (int16,
      128B elems at 256B stride, 1024 idx/call = HW ring limit), then per
      chunk a one-hot matmul gt.T @ oh accumulates the TRANSPOSED block
      (hid, dest) in PSUM; cells evict via one Activation copy into fp16
      partial tables.
    Partial tables are split 4 ways per dest shard [5,4,3,2 cells]; the
      ReduceScatter(add) of each split is issued mid-scatter so it
      overlaps the remaining cells, and phase D (tab2 = relu(h1).T @ W2,
      no transpose needed since partials are stored transposed) is
      interleaved into the scatter loop as each split's reduction lands.
    L2 scatter identical with 64-wide fp16 rows (128B gather elems);
      final f32 output is PE-transposed from the reduced shard.
  One-hot build: cell rows are value-coded 0..895 so chunks can span
  blocks (is_equal vs per-block iota slices masks everything else,
  PADVAL=1000 kills padding); rowloc is stored duplicated x2 and viewed
  with a packed 4-dim AP so the TensorTensor qualifies for the DVE 2x_1p
  fast mode. Gather payloads below 256B bypass bass's transpose-only
  assert via _dma_gather_raw (ucode allows them for non-transpose).
"""
import sys
sys.path.insert(0, "/opt/trn_rl_repo")

import numpy as np
from contextlib import ExitStack

import concourse.bass as bass
import concourse.bacc as bacc
import concourse.tile as tile
from concourse import bass_utils
from concourse import mybir
from concourse.library_config import mlp

PADVAL = 1000.0
GMAX = 8          # 128-idx chunks per dma_gather call (HW max: 1024 idx)
SCRATCH = 16384   # dynamic_dma_scratch_size (bytes/partition)
CELLR = 896       # dest rows per cell (7 blocks of 128)
NBLK = CELLR // 128
CPK = 14          # cells per dest shard (NPP // CELLR)
SPLITS = [0, 5, 9, 12, 13, 14]   # RS split bounds (cells per shard)
NSPL = 5


class Config:
    def __init__(self, n=100000, in_dim=256, hid=128, out_dim=64, ncore=8):
        self.N = n
        self.IN = in_dim
        self.HID = hid
        self.OUT = out_dim
        self.NCORE = ncore
        self.NPC = n // ncore
        self.NB = (self.NPC + 127) // 128
        self.NPP = self.NB * 128
        self.NT = ncore * self.NPP
        self.NCELL = self.NT // CELLR
        self.KT = in_dim // 128
        self.SROWS = [(SPLITS[s + 1] - SPLITS[s]) * CELLR
                      for s in range(NSPL)]


CFG = Config()
CELL_SEQ = [k * CPK + loc
            for s in range(NSPL)
            for k in range(CFG.NCORE)
            for loc in range(SPLITS[s], SPLITS[s + 1])]
END_POS = [CFG.NCORE * SPLITS[s + 1] - 1 for s in range(NSPL)]


def prep_edges(cfg, edge_index):
    """Bucket edges by (src core, dest cell); A-block edges at the cell
    start, B-block edges packed at the tail. Chunk counts per cell are
    uniform across cores (max). Cells are laid out in CELL_SEQ order."""
    c = cfg
    row = np.asarray(edge_index[0], dtype=np.int64)
    col = np.asarray(edge_index[1], dtype=np.int64)
    src = col // c.NPC
    tloc = (col - src * c.NPC).astype(np.int16)
    rT = (row // c.NPC) * c.NPP + (row % c.NPC)
    cell = rT // CELLR
    par = (rT // 128) % NBLK
    rib = (rT % CELLR).astype(np.float16)  # 0..CELLR-1, parity-coded

    key = (src * c.NCELL + cell) * NBLK + par
    order = np.argsort(key, kind="stable")
    tloc_s = tloc[order]
    rib_s = rib[order]
    cnt = np.bincount(
        key[order], minlength=c.NCORE * c.NCELL * NBLK).reshape(
        c.NCORE, c.NCELL, NBLK)
    starts = np.concatenate([[0], np.cumsum(cnt.reshape(-1))])
    tot = cnt.sum(axis=2)

    m_cell = np.maximum(-(-tot.max(axis=0) // 128), 1)
    # prefix sums S_i per (core, cell); compile-time block spans
    S = np.zeros((c.NCORE, c.NCELL, NBLK + 1), np.int64)
    S[:, :, 1:] = np.cumsum(cnt, axis=2)
    blo = np.zeros((c.NCELL, NBLK), np.int64)
    bhi = np.zeros((c.NCELL, NBLK), np.int64)
    for i in range(NBLK - 1):
        blo[:, i] = S[:, :, i].min(axis=0) // 128
        bhi[:, i] = -(-S[:, :, i + 1].max(axis=0) // 128)
    blo[:, NBLK - 1] = m_cell - np.maximum(
        -(-cnt[:, :, NBLK - 1].max(axis=0) // 128), 1)
    bhi[:, NBLK - 1] = m_cell
    for i in range(NBLK):
        blo[:, i] = np.minimum(blo[:, i], m_cell - 1)
        bhi[:, i] = np.clip(bhi[:, i], blo[:, i] + 1, m_cell)
    qoff = np.zeros(c.NCELL, np.int64)
    q = 0
    for ce in CELL_SEQ:
        qoff[ce] = q
        q += int(m_cell[ce])
    nch = int(q)

    colidx = np.zeros((c.NCORE, 128, nch * 8), np.int16)
    rowloc2 = np.full((c.NCORE, 128, nch * 2), PADVAL, np.float16)

    for cc in range(c.NCORE):
        for ce in range(c.NCELL):
            nk = int(m_cell[ce])
            cap = nk * 128
            ki = (cc * c.NCELL + ce) * NBLK
            sh0, shE = starts[ki], starts[ki + NBLK - 1]  # head blocks
            sT = starts[ki + NBLK]                        # tail end
            nh = int(shE - sh0)
            nt = int(sT - shE)
            tl = np.zeros(cap, np.int16)
            tl[:nh] = tloc_s[sh0:shE]
            tl[cap - nt:] = tloc_s[shE:sT]
            rb = np.full(cap, PADVAL, np.float16)
            rb[:nh] = rib_s[sh0:shE]
            rb[cap - nt:] = rib_s[shE:sT]
            qo = int(qoff[ce])
            colidx[cc][:, qo * 8:(qo + nk) * 8] = np.tile(
                tl.reshape(-1, 16).T, (8, 1))
            rowloc2[cc][:, qo * 2:(qo + nk) * 2] = np.repeat(
                rb.reshape(nk, 128).T, 2, axis=1)
    meta = dict(m_cell=m_cell, blo=blo, bhi=bhi, qoff=qoff, nch=nch)
    return colidx, rowloc2, meta


def _dma_gather_raw(gp, out_ap, in_ap, idxs_ap, num_idxs, num_idxs_reg,
                    elem_size, elem_step, queue_num):
    """dma_gather (non-transpose, DRAM source) without the %256 payload
    assert — the ucode only requires 256B multiples for transpose mode;
    the source ROW STRIDE must still be a 256B multiple."""
    gp._assert_queue_num(queue_num)
    assert idxs_ap.dtype == mybir.dt.int16
    assert in_ap.dtype == out_ap.dtype
    elem_size_bytes = elem_size * mybir.dt.size(in_ap.dtype)
    assert elem_size_bytes > 0 and elem_size_bytes % 128 == 0
    assert in_ap.space == bass.MemorySpace.DRAM
    assert idxs_ap.space == bass.MemorySpace.SBUF
    assert out_ap.space == bass.MemorySpace.SBUF
    assert in_ap.ap[-1][1] == out_ap.ap[-1][1] == elem_size
    assert out_ap.ap[0][1] * out_ap.ap[1][1] == ((num_idxs + 127) // 128) * 128
    assert in_ap.ap[0][0] == elem_step
    stride_bytes = elem_step * mybir.dt.size(in_ap.dtype)
    stride_bytes_256 = stride_bytes // 256
    assert stride_bytes_256 * 256 == stride_bytes and stride_bytes_256 < 256
    _in_ap = gp.lower_ap_dma(in_ap, for_custom_bir_dma=True)
    _idxs_ap = gp.lower_ap(idxs_ap)
    _out_ap = gp.lower_ap(out_ap)
    return gp.add_instruction(
        mybir.InstDMAGatherAnt(
            name=gp.bass.get_next_instruction_name(),
            ins=[*_in_ap, _idxs_ap,
                 gp.lower_val_access(gp.to_reg(num_idxs_reg))],
            outs=[_out_ap],
            transpose=False, num_idxs=num_idxs, elem_size=elem_size,
            stride_bytes_256=stride_bytes_256, gen_mode=0,
            single_packet=True, queue_num=queue_num,
            sbuf_tokens_per_rank=0, sbuf_free_dim_per_rank=0,
            sbuf_free_dim_pad_per_rank=0, sbuf_byte_offset=0))


def build(cfg, meta):
    c = cfg
    m_cell, blo, bhi, qoff, nch = (
        meta["m_cell"], meta["blo"], meta["bhi"], meta["qoff"], meta["nch"])
    nc = bacc.Bacc(None, target_bir_lowering=False, debug=False,
                   num_devices=c.NCORE, name="gcn3", num_swdge_queues=4,
                   dynamic_dma_scratch_size=SCRATCH)
    f16, f32, i16 = mybir.dt.float16, mybir.dt.float32, mybir.dt.int16
    iseq = mybir.AluOpType.is_equal
    COPY = mybir.ActivationFunctionType.Copy

    f8 = mybir.dt.float8e4
    xT = nc.dram_tensor("xT", (c.IN, c.NPP), f16, kind="ExternalInput")
    w1 = nc.dram_tensor("w1", (c.IN, c.HID), f16, kind="ExternalInput")
    w2 = nc.dram_tensor("w2", (c.HID, c.OUT), f16, kind="ExternalInput")
    ident = nc.dram_tensor("ident", (128, 128), f16, kind="ExternalInput")
    iotaAB = nc.dram_tensor("iotaAB", (128, CELLR), f16,
                            kind="ExternalInput")
    colidx = nc.dram_tensor("colidx", (128, nch * 8), i16,
                            kind="ExternalInput")
    rowloc2 = nc.dram_tensor("rowloc2", (128, nch * 2), f16,
                             kind="ExternalInput")
    out = nc.dram_tensor("out", (c.NPC, c.OUT), f32, kind="ExternalOutput")

    tab1 = nc.dram_tensor("tab1", (c.NPP, 2 * c.HID), f8, kind="Internal")
    tab2 = nc.dram_tensor("tab2", (c.NPP, 128), f16, kind="Internal")
    p1s = [nc.dram_tensor(f"p1s{s}", (c.NCORE, c.HID, c.SROWS[s]), f16,
                          kind="Internal") for s in range(NSPL)]
    r1s = [nc.dram_tensor(f"r1s{s}", (c.HID, c.SROWS[s]), f16,
                          kind="Internal") for s in range(NSPL)]
    p2s = [nc.dram_tensor(f"p2s{s}", (c.NCORE, c.OUT, c.SROWS[s]), f16,
                          kind="Internal") for s in range(NSPL)]
    r2s = [nc.dram_tensor(f"r2s{s}", (c.OUT, c.SROWS[s]), f16,
                          kind="Internal") for s in range(NSPL)]

    groups = [list(range(c.NCORE))]

    with ExitStack() as ctx:
        tc = ctx.enter_context(tile.TileContext(nc))
        nc.gpsimd.load_library(mlp)
        cpool = ctx.enter_context(tc.tile_pool(name="const", bufs=1))
        ident_sb = cpool.tile((128, 128), f16, tag="ident")
        nc.sync.dma_start(ident_sb[:], ident[:])
        iota_sb = cpool.tile((128, CELLR), f16, tag="iota")
        nc.sync.dma_start(iota_sb[:], iotaAB[:])
        w2_sb = cpool.tile((c.HID, c.OUT), f16, tag="w2")
        nc.sync.dma_start(w2_sb[:], w2[:])
        rl2_sb = cpool.tile((128, nch * 2), f16, tag="rl2")
        rl2_loaded = [False]
        segb = [0, 32 * GMAX]
        while segb[-1] < nch:
            segb.append(min(segb[-1] + 128 * GMAX, nch))
        seg0_tile = cpool.tile((128, 32 * GMAX * 8), i16, tag="seg0")
        nc.sync.dma_start(seg0_tile[:], colidx.ap()[:, 0:32 * GMAX * 8])

        # Phase A: tab1 = fp8(X @ W1), 4 blocks per DMA
        with tc.tile_pool(name="pa", bufs=1) as pa, \
             tc.tile_pool(name="pas", bufs=5) as pas, \
             tc.tile_pool(name="psa", bufs=6, space="PSUM") as psa:
            w1k = []
            hN = c.NPP // 2
            for k in range(c.KT):
                t = pa.tile((128, c.HID), f16, tag=f"w{k}", name=f"w1k{k}")
                nc.sync.dma_start(t[:], w1.ap()[k * 128:(k + 1) * 128, :])
                w1k.append(t)
            xkh = {}
            for half in range(2):
                for k in range(c.KT):
                    t = pa.tile((128, hN), f16, tag=f"x{k}h{half}",
                                name=f"xk{k}h{half}")
                    nc.sync.dma_start(
                        t[:], xT.ap()[k * 128:(k + 1) * 128,
                                      half * hN:(half + 1) * hN])
                    xkh[(k, half)] = t
            s1g = None
            ps = None
            for b in range(c.NB):
                q = b % 4
                if q % 2 == 0:
                    ps = psa.tile((128, 2 * c.HID), f32, space="PSUM",
                                  name="ps", tag="ps")
                sl = ps[:, (q % 2) * c.HID:(q % 2) * c.HID + c.HID]
                half, boff = (0, b) if b * 128 < hN else (1, b - hN // 128)
                for k in range(c.KT):
                    nc.tensor.matmul(sl,
                                     xkh[(k, half)][:, boff * 128:
                                                    (boff + 1) * 128],
                                     w1k[k][:], start=(k == 0),
                                     stop=(k == c.KT - 1))
                if q == 0:
                    s1g = pas.tile((128, 4, c.HID), f8, name="s1g",
                                   tag="s1g")
                if q % 2 == 1 or b == c.NB - 1:
                    w = (q % 2 + 1) * c.HID
                    dv = s1g[:, q - q % 2:q + 1, :]
                    dvv = bass.AP(dv.tensor, dv.offset,
                                  [dv.ap[0], [1, w]])
                    if (b // 2) % 2 == 0:
                        nc.scalar.activation(dvv, ps[:, 0:w], COPY)
                    else:
                        nc.vector.tensor_copy(dvv, ps[:, 0:w])
                if q == 3 or b == c.NB - 1:
                    b0 = b - q
                    dst = bass.AP(tab1, b0 * 128 * 2 * c.HID,
                                  [[2 * c.HID, 128],
                                   [128 * 2 * c.HID, q + 1], [1, c.HID]])
                    nc.sync.dma_start(dst, s1g[:, 0:q + 1, :])

        # Phase D group: tab2 rows [.] = relu(h1).T @ W2 (rs pre-transposed)
        def d_group(pd, psd, rsrc, boff, b0, nb):
            h = pd.tile((c.HID, nb * 128), f16, tag="h", name="h")
            nc.sync.dma_start(
                h[:], rsrc.ap()[:, b0 * 128:(b0 + nb) * 128])
            hr = pd.tile((c.HID, nb * 128), f16, tag="hr", name="hr")
            nc.vector.tensor_scalar_max(hr[:], h[:], 0.0)
            s2 = pd.tile((128, nb, c.OUT), f16, tag="s2", name="s2")
            for q in range(nb):
                ps2 = psd.tile((128, c.OUT), f32, space="PSUM")
                nc.tensor.matmul(ps2[:], hr[:, q * 128:(q + 1) * 128],
                                 w2_sb[:], start=True, stop=True)
                nc.scalar.activation(s2[:, q, :], ps2[:], COPY)
            dst = bass.AP(tab2, (boff + b0) * 128 * 128,
                          [[128, 128], [128 * 128, nb], [1, c.OUT]])
            nc.sync.dma_start(dst, s2[:])

        # Phase F group: out rows = transpose(rs2 block) as f32
        def f_group(pf, psf, rsrc, boff, b0, nb):
            t = pf.tile((c.OUT, nb * 128), f16, tag="t", name="t")
            nc.sync.dma_start(
                t[:], rsrc.ap()[:, b0 * 128:(b0 + nb) * 128])
            of = pf.tile((128, nb, c.OUT), f32, tag="of", name="of")
            for q in range(nb):
                tp = psf.tile((128, c.OUT), f16, space="PSUM")
                nc.tensor.transpose(out=tp[:],
                                    in_=t[:, q * 128:(q + 1) * 128],
                                    identity=ident_sb[0:c.OUT, 0:c.OUT])
                nc.vector.tensor_copy(of[:, q, :], tp[:])
            gb = boff + b0
            rows = min(nb * 128, c.NPC - gb * 128)
            nfull = rows // 128
            if nfull > 0:
                dst = bass.AP(out, gb * 128 * c.OUT,
                              [[c.OUT, 128], [128 * c.OUT, nfull],
                               [1, c.OUT]])
                nc.sync.dma_start(dst, of[:, 0:nfull, :])
            rem = rows - nfull * 128
            if rem > 0:
                nc.sync.dma_start(
                    out.ap()[(gb + nfull) * 128:(gb + nfull) * 128 + rem, :],
                    of[0:rem, nfull, :])

        def scatter_layer(tab_ap, elem, estep, W, parts, gdt,
                          mid_cbs=(), job_lists=()):
            """Gather+scatter all dest cells in CELL_SEQ order. mid_cbs[s]
            fires shortly after split s completes; job_lists[s] are
            interleaved afterwards (one per cell)."""
            gq = [0]
            call_tile = {}
            seg_tile = {0: seg0_tile}
            if not rl2_loaded[0]:
                rl2_loaded[0] = True
                nc.sync.dma_start(rl2_sb[:], rowloc2[:])
            cb_at = {min(END_POS[s] + 10, len(CELL_SEQ) - 2): cb
                     for s, cb in enumerate(mid_cbs)}
            jobs_at = {END_POS[s] + 34: list(j)
                       for s, j in enumerate(job_lists)}
            live_jobs = []

            with tc.tile_pool(name="poh", bufs=10) as poh, \
                 tc.tile_pool(name="pix", bufs=2) as pix, \
                 tc.tile_pool(name="pg", bufs=22) as pg, \
                 tc.tile_pool(name="pem", bufs=5) as pem, \
                 tc.tile_pool(name="pso", bufs=2, space="PSUM") as pso:

                def ensure_call(qc):
                    if qc in call_tile:
                        return call_tile[qc]
                    ch = qc * GMAX
                    s = next(i for i in range(len(segb) - 1)
                             if ch < segb[i + 1])
                    if s not in seg_tile:
                        ch0, ch1 = segb[s], segb[s + 1]
                        st = pix.tile((128, (ch1 - ch0) * 8), i16,
                                      name="st", tag="st")
                        nc.sync.dma_start(
                            st[:], colidx.ap()[:, ch0 * 8:ch1 * 8])
                        seg_tile[s] = st
                    st = seg_tile[s]
                    sn = min(GMAX, nch - qc * GMAX)
                    off = (qc * GMAX - segb[s]) * 8
                    gt = pg.tile((128, sn, elem), gdt, name="gt", tag="gt")
                    _dma_gather_raw(
                        nc.gpsimd, gt[:], tab_ap, st[:, off:off + sn * 8],
                        sn * 128, sn * 128, elem, estep, gq[0] % 4)
                    gq[0] += 1
                    call_tile[qc] = (gt, sn)
                    return call_tile[qc]

                for pos, ce in enumerate(CELL_SEQ):
                    k = ce // CPK
                    loc = ce % CPK
                    sp = next(s for s in range(NSPL)
                              if loc < SPLITS[s + 1])
                    lh = loc - SPLITS[sp]
                    cb = int(qoff[ce])
                    m = int(m_cell[ce])
                    los = [int(blo[ce][i]) for i in range(NBLK)]
                    his = [int(bhi[ce][i]) for i in range(NBLK)]
                    cbase = [0]
                    for i in range(NBLK):
                        cbase.append(cbase[-1] + his[i] - los[i])
                    ncols = cbase[-1]
                    oh = poh.tile((128, ncols, 128), f16, name="oh",
                                  tag="oh")
                    for i in range(NBLK):
                        nco = his[i] - los[i]
                        ox = oh[:, cbase[i]:cbase[i + 1], :]
                        ov = bass.AP(ox.tensor, ox.offset,
                                     [ox.ap[0], [128, nco], [2, 64], [1, 2]])
                        ix = iota_sb[:, i * 128:(i + 1) * 128]
                        iv = bass.AP(ix.tensor, ix.offset,
                                     [ix.ap[0], [0, nco], [2, 64], [1, 2]])
                        j0 = cb + los[i]
                        rx = rl2_sb[:, 2 * j0:2 * j0 + 2 * nco]
                        rv = bass.AP(rx.tensor, rx.offset,
                                     [rx.ap[0], [2, nco], [0, 64], [1, 2]])
                        nc.vector.tensor_tensor(out=ov, in0=iv, in1=rv,
                                                op=iseq)
                    psU = pso.tile((W, 512), f32, space="PSUM",
                                   name="psU", tag="psU")
                    psV = pso.tile((W, 384), f32, space="PSUM",
                                   name="psV", tag="psV")
                    for i in range(NBLK):
                        pp = (psU[:, (i % 4) * 128:(i % 4) * 128 + 128]
                              if i < 4 else
                              psV[:, (i - 4) * 128:(i - 4) * 128 + 128])
                        for j in range(los[i], his[i]):
                            gi = cb + j
                            gt, sn = ensure_call(gi // GMAX)
                            sl = gt[:, gi % GMAX, 0:W]
                            nc.tensor.matmul(pp, sl,
                                             oh[:, cbase[i] + j - los[i], :],
                                             start=(j == los[i]),
                                             stop=(j == his[i] - 1))
                    emU = pem.tile((W, 512), f16, name="emU", tag="emU")
                    nc.scalar.activation(emU[:], psU[:], COPY)
                    emV = pem.tile((W, 384), f16, name="emV", tag="emV")
                    nc.scalar.activation(emV[:], psV[:], COPY)
                    i0 = lh * CELLR
                    nc.sync.dma_start(
                        parts[sp].ap()[k, :, i0:i0 + 512], emU[:])
                    nc.sync.dma_start(
                        parts[sp].ap()[k, :, i0 + 512:i0 + 896], emV[:])
                    if pos in cb_at:
                        cb_at.pop(pos)()
                    if pos in jobs_at:
                        live_jobs.extend(jobs_at.pop(pos))
                    if live_jobs:
                        live_jobs.pop(0)()
                for p in sorted(cb_at):
                    cb_at.pop(p)()
                for p in sorted(jobs_at):
                    live_jobs.extend(jobs_at.pop(p))
                while live_jobs:
                    live_jobs.pop(0)()

        # ---- Layer 1 ----
        pd = ctx.enter_context(tc.tile_pool(name="pd", bufs=3))
        psd = ctx.enter_context(tc.tile_pool(name="psd", bufs=2,
                                             space="PSUM"))
        sblk = [r // 128 for r in c.SROWS]          # blocks per split
        soff = [sum(sblk[:s]) for s in range(NSPL)]  # block offsets

        def rs_cb(ps, rs, s):
            def cb():
                nc.gpsimd.collective_compute(
                    "ReduceScatter", mybir.AluOpType.add,
                    replica_groups=groups,
                    ins=[ps[s].ap()], outs=[rs[s].ap()])
            return cb

        def grp_jobs(fn, pools, rs, s):
            return [(lambda b0=b0: fn(pools[0], pools[1], rs[s], soff[s],
                                      b0, min(4, sblk[s] - b0)))
                    for b0 in range(0, sblk[s], 4)]

        scatter_layer(
            tab1.ap()[:, 0:c.HID], c.HID, 2 * c.HID, c.HID, p1s, f8,
            mid_cbs=[rs_cb(p1s, r1s, s) for s in range(NSPL - 1)],
            job_lists=[grp_jobs(d_group, (pd, psd), r1s, s)
                       for s in range(NSPL - 1)])
        rs_cb(p1s, r1s, NSPL - 1)()
        for j in grp_jobs(d_group, (pd, psd), r1s, NSPL - 1):
            j()

        # ---- Layer 2 ----
        pf = ctx.enter_context(tc.tile_pool(name="pf", bufs=3))
        psf = ctx.enter_context(tc.tile_pool(name="psf", bufs=2,
                                             space="PSUM"))
        scatter_layer(
            tab2.ap()[:, 0:c.OUT], c.OUT, 128, c.OUT, p2s, f16,
            mid_cbs=[rs_cb(p2s, r2s, s) for s in range(NSPL - 1)],
            job_lists=[grp_jobs(f_group, (pf, psf), r2s, s)
                       for s in range(NSPL - 1)])
        rs_cb(p2s, r2s, NSPL - 1)()
        for j in grp_jobs(f_group, (pf, psf), r2s, NSPL - 1):
            j()

    nc.compile()
    return nc


def make_inputs(cfg, features, edge_index, W1, W2):
    c = cfg
    colidx, rowloc2, meta = prep_edges(cfg, edge_index)
    iota2d = np.broadcast_to(np.arange(CELLR, dtype=np.float16),
                             (128, CELLR)).copy()
    ident = np.eye(128, dtype=np.float16)
    w1 = np.ascontiguousarray(np.asarray(W1, np.float16))
    w2 = np.ascontiguousarray(np.asarray(W2, np.float16))
    in_maps = []
    for cc in range(c.NCORE):
        xc = np.asarray(features[cc * c.NPC:(cc + 1) * c.NPC], np.float32)
        xt = np.zeros((c.IN, c.NPP), np.float16)
        xt[:, :c.NPC] = xc.T.astype(np.float16)
        in_maps.append({
            "xT": np.ascontiguousarray(xt),
            "w1": w1, "w2": w2, "ident": ident, "iotaAB": iota2d,
            "colidx": np.ascontiguousarray(colidx[cc]),
            "rowloc2": np.ascontiguousarray(rowloc2[cc]),
        })
    return in_maps, meta


_LAST_NC = None


def kernel(features, edge_index, W1, W2):
    global _LAST_NC
    cfg = CFG
    in_maps, meta = make_inputs(cfg, features, edge_index, W1, W2)
    nc = build(cfg, meta)
    _LAST_NC = nc
    res = bass_utils.run_bass_kernel_spmd(
        nc, in_maps, core_ids=list(range(cfg.NCORE)))
    return np.concatenate(
        [res.results[cc]["out"] for cc in range(cfg.NCORE)], axis=0)


# revision 59
# speedup vs baseline: 1.0064x; 1.0064x over previous
"""2-layer GCN (gnn_message_passing) on 8 Trainium2 NeuronCores.

Source-sharded scatter with split ReduceScatter collectives:
  Edges live on the core owning the SOURCE node (col); dest rows are
  grouped into 896-row cells (7 blocks of 128) over the padded global
  row space. Per core:
    tab1 = fp8e4(X_local @ W1)   (12544 rows, 256B stride, DRAM)
    L1 scatter, per cell: dma_gather table rows by local col idx (int16,
      128B elems at 256B stride, 1024 idx/call = HW ring limit), then per
      chunk a one-hot matmul gt.T @ oh accumulates the TRANSPOSED block
      (hid, dest) in PSUM; cells evict via one Activation copy into fp16
      partial tables.
    Partial tables are split 4 ways per dest shard [5,4,3,2 cells]; the
      ReduceScatter(add) of each split is issued mid-scatter so it
      overlaps the remaining cells, and phase D (tab2 = relu(h1).T @ W2,
      no transpose needed since partials are stored transposed) is
      interleaved into the scatter loop as each split's reduction lands.
    L2 scatter identical with 64-wide fp16 rows (128B gather elems);
      final f32 output is PE-transposed from the reduced shard.
  One-hot build: cell rows are value-coded 0..895 so chunks can span
  blocks (is_equal vs per-block iota slices masks everything else,
  PADVAL=1000 kills padding); rowloc is stored duplicated x2 and viewed
  with a packed 4-dim AP so the TensorTensor qualifies for the DVE 2x_1p
  fast mode. Gather payloads below 256B bypass bass's transpose-only
  assert via _dma_gather_raw (ucode allows them for non-transpose).
"""
import sys
sys.path.insert(0, "/opt/trn_rl_repo")

import numpy as np
from contextlib import ExitStack

import concourse.bass as bass
import concourse.bacc as bacc
import concourse.tile as tile
from concourse import bass_utils
from concourse import mybir
from concourse.library_config import mlp

PADVAL = 1000.0
GMAX = 8          # 128-idx chunks per dma_gather call (HW max: 1024 idx)
SCRATCH = 16384   # dynamic_dma_scratch_size (bytes/partition)
CELLR = 896       # dest rows per cell (7 blocks of 128)
NBLK = CELLR // 128
CPK = 14          # cells per dest shard (NPP // CELLR)
SPLITS = [0, 5, 9, 12, 13, 14]   # RS split bounds (cells per shard)
NSPL = 5


class Config:
    def __init__(self, n=100000, in_dim=256, hid=128, out_dim=64, ncore=8):
        self.N = n
        self.IN = in_dim
        self.HID = hid
        self.OUT = out_dim
        self.NCORE = ncore
        self.NPC = n // ncore
        self.NB = (self.NPC + 127) // 128
        self.NPP = self.NB * 128
        self.NT = ncore * self.NPP
        self.NCELL = self.NT // CELLR
        self.KT = in_dim // 128
        self.SROWS = [(SPLITS[s + 1] - SPLITS[s]) * CELLR
                      for s in range(NSPL)]


CFG = Config()
CELL_SEQ = [k * CPK + loc
            for s in range(NSPL)
            for k in range(CFG.NCORE)
            for loc in range(SPLITS[s], SPLITS[s + 1])]
END_POS = [CFG.NCORE * SPLITS[s + 1] - 1 for s in range(NSPL)]


def prep_edges(cfg, edge_index):
    """Bucket edges by (src core, dest cell); A-block edges at the cell
    start, B-block edges packed at the tail. Chunk counts per cell are
    uniform across cores (max). Cells are laid out in CELL_SEQ order."""
    c = cfg
    row = np.asarray(edge_index[0], dtype=np.int64)
    col = np.asarray(edge_index[1], dtype=np.int64)
    src = col // c.NPC
    tloc = (col - src * c.NPC).astype(np.int16)
    rT = (row // c.NPC) * c.NPP + (row % c.NPC)
    cell = rT // CELLR
    par = (rT // 128) % NBLK
    rib = (rT % CELLR).astype(np.float16)  # 0..CELLR-1, parity-coded

    key = (src * c.NCELL + cell) * NBLK + par
    order = np.argsort(key, kind="stable")
    tloc_s = tloc[order]
    rib_s = rib[order]
    cnt = np.bincount(
        key[order], minlength=c.NCORE * c.NCELL * NBLK).reshape(
        c.NCORE, c.NCELL, NBLK)
    starts = np.concatenate([[0], np.cumsum(cnt.reshape(-1))])
    tot = cnt.sum(axis=2)

    m_cell = np.maximum(-(-tot.max(axis=0) // 128), 1)
    # prefix sums S_i per (core, cell); compile-time block spans
    S = np.zeros((c.NCORE, c.NCELL, NBLK + 1), np.int64)
    S[:, :, 1:] = np.cumsum(cnt, axis=2)
    blo = np.zeros((c.NCELL, NBLK), np.int64)
    bhi = np.zeros((c.NCELL, NBLK), np.int64)
    for i in range(NBLK - 1):
        blo[:, i] = S[:, :, i].min(axis=0) // 128
        bhi[:, i] = -(-S[:, :, i + 1].max(axis=0) // 128)
    blo[:, NBLK - 1] = m_cell - np.maximum(
        -(-cnt[:, :, NBLK - 1].max(axis=0) // 128), 1)
    bhi[:, NBLK - 1] = m_cell
    for i in range(NBLK):
        blo[:, i] = np.minimum(blo[:, i], m_cell - 1)
        bhi[:, i] = np.clip(bhi[:, i], blo[:, i] + 1, m_cell)
    qoff = np.zeros(c.NCELL, np.int64)
    q = 0
    for ce in CELL_SEQ:
        qoff[ce] = q
        q += int(m_cell[ce])
    nch = int(q)

    colidx = np.zeros((c.NCORE, 128, nch * 8), np.int16)
    rowloc2 = np.full((c.NCORE, 128, nch * 2), PADVAL, np.float16)

    for cc in range(c.NCORE):
        for ce in range(c.NCELL):
            nk = int(m_cell[ce])
            cap = nk * 128
            ki = (cc * c.NCELL + ce) * NBLK
            sh0, shE = starts[ki], starts[ki + NBLK - 1]  # head blocks
            sT = starts[ki + NBLK]                        # tail end
            nh = int(shE - sh0)
            nt = int(sT - shE)
            tl = np.zeros(cap, np.int16)
            tl[:nh] = tloc_s[sh0:shE]
            tl[cap - nt:] = tloc_s[shE:sT]
            rb = np.full(cap, PADVAL, np.float16)
            rb[:nh] = rib_s[sh0:shE]
            rb[cap - nt:] = rib_s[shE:sT]
            qo = int(qoff[ce])
            colidx[cc][:, qo * 8:(qo + nk) * 8] = np.tile(
                tl.reshape(-1, 16).T, (8, 1))
            rowloc2[cc][:, qo * 2:(qo + nk) * 2] = np.repeat(
                rb.reshape(nk, 128).T, 2, axis=1)
    meta = dict(m_cell=m_cell, blo=blo, bhi=bhi, qoff=qoff, nch=nch)
    return colidx, rowloc2, meta


def _dma_gather_raw(gp, out_ap, in_ap, idxs_ap, num_idxs, num_idxs_reg,
                    elem_size, elem_step, queue_num):
    """dma_gather (non-transpose, DRAM source) without the %256 payload
    assert — the ucode only requires 256B multiples for transpose mode;
    the source ROW STRIDE must still be a 256B multiple."""
    gp._assert_queue_num(queue_num)
    assert idxs_ap.dtype == mybir.dt.int16
    assert in_ap.dtype == out_ap.dtype
    elem_size_bytes = elem_size * mybir.dt.size(in_ap.dtype)
    assert elem_size_bytes > 0 and elem_size_bytes % 128 == 0
    assert in_ap.space == bass.MemorySpace.DRAM
    assert idxs_ap.space == bass.MemorySpace.SBUF
    assert out_ap.space == bass.MemorySpace.SBUF
    assert in_ap.ap[-1][1] == out_ap.ap[-1][1] == elem_size
    assert out_ap.ap[0][1] * out_ap.ap[1][1] == ((num_idxs + 127) // 128) * 128
    assert in_ap.ap[0][0] == elem_step
    stride_bytes = elem_step * mybir.dt.size(in_ap.dtype)
    stride_bytes_256 = stride_bytes // 256
    assert stride_bytes_256 * 256 == stride_bytes and stride_bytes_256 < 256
    _in_ap = gp.lower_ap_dma(in_ap, for_custom_bir_dma=True)
    _idxs_ap = gp.lower_ap(idxs_ap)
    _out_ap = gp.lower_ap(out_ap)
    return gp.add_instruction(
        mybir.InstDMAGatherAnt(
            name=gp.bass.get_next_instruction_name(),
            ins=[*_in_ap, _idxs_ap,
                 gp.lower_val_access(gp.to_reg(num_idxs_reg))],
            outs=[_out_ap],
            transpose=False, num_idxs=num_idxs, elem_size=elem_size,
            stride_bytes_256=stride_bytes_256, gen_mode=0,
            single_packet=True, queue_num=queue_num,
            sbuf_tokens_per_rank=0, sbuf_free_dim_per_rank=0,
            sbuf_free_dim_pad_per_rank=0, sbuf_byte_offset=0))


def build(cfg, meta):
    c = cfg
    m_cell, blo, bhi, qoff, nch = (
        meta["m_cell"], meta["blo"], meta["bhi"], meta["qoff"], meta["nch"])
    nc = bacc.Bacc(None, target_bir_lowering=False, debug=False,
                   num_devices=c.NCORE, name="gcn3", num_swdge_queues=4,
                   dynamic_dma_scratch_size=SCRATCH)
    f16, f32, i16 = mybir.dt.float16, mybir.dt.float32, mybir.dt.int16
    iseq = mybir.AluOpType.is_equal
    COPY = mybir.ActivationFunctionType.Copy

    f8 = mybir.dt.float8e4
    xT = nc.dram_tensor("xT", (c.IN, c.NPP), f16, kind="ExternalInput")
    w1 = nc.dram_tensor("w1", (c.IN, c.HID), f16, kind="ExternalInput")
    w2 = nc.dram_tensor("w2", (c.HID, c.OUT), f16, kind="ExternalInput")
    ident = nc.dram_tensor("ident", (128, 128), f16, kind="ExternalInput")
    iotaAB = nc.dram_tensor("iotaAB", (128, CELLR), f16,
                            kind="ExternalInput")
    colidx = nc.dram_tensor("colidx", (128, nch * 8), i16,
                            kind="ExternalInput")
    rowloc2 = nc.dram_tensor("rowloc2", (128, nch * 2), f16,
                             kind="ExternalInput")
    out = nc.dram_tensor("out", (c.NPC, c.OUT), f32, kind="ExternalOutput")

    tab1 = nc.dram_tensor("tab1", (c.NPP, 2 * c.HID), f8, kind="Internal")
    tab2 = nc.dram_tensor("tab2", (c.NPP, 128), f16, kind="Internal")
    p1s = [nc.dram_tensor(f"p1s{s}", (c.NCORE, c.HID, c.SROWS[s]), f16,
                          kind="Internal") for s in range(NSPL)]
    r1s = [nc.dram_tensor(f"r1s{s}", (c.HID, c.SROWS[s]), f16,
                          kind="Internal") for s in range(NSPL)]
    p2s = [nc.dram_tensor(f"p2s{s}", (c.NCORE, c.OUT, c.SROWS[s]), f16,
                          kind="Internal") for s in range(NSPL)]
    r2s = [nc.dram_tensor(f"r2s{s}", (c.OUT, c.SROWS[s]), f16,
                          kind="Internal") for s in range(NSPL)]

    groups = [list(range(c.NCORE))]

    with ExitStack() as ctx:
        tc = ctx.enter_context(tile.TileContext(nc))
        nc.gpsimd.load_library(mlp)
        cpool = ctx.enter_context(tc.tile_pool(name="const", bufs=1))
        ident_sb = cpool.tile((128, 128), f16, tag="ident")
        nc.sync.dma_start(ident_sb[:], ident[:])
        iota_sb = cpool.tile((128, CELLR), f16, tag="iota")
        nc.sync.dma_start(iota_sb[:], iotaAB[:])
        w2_sb = cpool.tile((c.HID, c.OUT), f16, tag="w2")
        nc.sync.dma_start(w2_sb[:], w2[:])
        rl2_sb = cpool.tile((128, nch * 2), f16, tag="rl2")
        rl2_loaded = [False]
        segb = [0, 32 * GMAX]
        while segb[-1] < nch:
            segb.append(min(segb[-1] + 128 * GMAX, nch))


        # Phase A: tab1 = fp8(X @ W1), 4 blocks per DMA
        with tc.tile_pool(name="pa", bufs=1) as pa, \
             tc.tile_pool(name="pas", bufs=5) as pas, \
             tc.tile_pool(name="psa", bufs=6, space="PSUM") as psa:
            w1k = []
            hN = c.NPP // 2
            for k in range(c.KT):
                t = pa.tile((128, c.HID), f16, tag=f"w{k}", name=f"w1k{k}")
                nc.sync.dma_start(t[:], w1.ap()[k * 128:(k + 1) * 128, :])
                w1k.append(t)
            xkh = {}
            for half in range(2):
                for k in range(c.KT):
                    t = pa.tile((128, hN), f16, tag=f"x{k}h{half}",
                                name=f"xk{k}h{half}")
                    nc.sync.dma_start(
                        t[:], xT.ap()[k * 128:(k + 1) * 128,
                                      half * hN:(half + 1) * hN])
                    xkh[(k, half)] = t
            s1g = None
            ps = None
            for b in range(c.NB):
                q = b % 4
                if q % 2 == 0:
                    ps = psa.tile((128, 2 * c.HID), f32, space="PSUM",
                                  name="ps", tag="ps")
                sl = ps[:, (q % 2) * c.HID:(q % 2) * c.HID + c.HID]
                half, boff = (0, b) if b * 128 < hN else (1, b - hN // 128)
                for k in range(c.KT):
                    nc.tensor.matmul(sl,
                                     xkh[(k, half)][:, boff * 128:
                                                    (boff + 1) * 128],
                                     w1k[k][:], start=(k == 0),
                                     stop=(k == c.KT - 1))
                if q == 0:
                    s1g = pas.tile((128, 4, c.HID), f8, name="s1g",
                                   tag="s1g")
                if q % 2 == 1 or b == c.NB - 1:
                    w = (q % 2 + 1) * c.HID
                    dv = s1g[:, q - q % 2:q + 1, :]
                    dvv = bass.AP(dv.tensor, dv.offset,
                                  [dv.ap[0], [1, w]])
                    if (b // 2) % 2 == 0:
                        nc.scalar.activation(dvv, ps[:, 0:w], COPY)
                    else:
                        nc.vector.tensor_copy(dvv, ps[:, 0:w])
                if q == 3 or b == c.NB - 1:
                    b0 = b - q
                    dst = bass.AP(tab1, b0 * 128 * 2 * c.HID,
                                  [[2 * c.HID, 128],
                                   [128 * 2 * c.HID, q + 1], [1, c.HID]])
                    nc.sync.dma_start(dst, s1g[:, 0:q + 1, :])

        # Phase D group: tab2 rows [.] = relu(h1).T @ W2 (rs pre-transposed)
        def d_group(pd, psd, rsrc, boff, b0, nb):
            h = pd.tile((c.HID, nb * 128), f16, tag="h", name="h")
            nc.sync.dma_start(
                h[:], rsrc.ap()[:, b0 * 128:(b0 + nb) * 128])
            hr = pd.tile((c.HID, nb * 128), f16, tag="hr", name="hr")
            nc.vector.tensor_scalar_max(hr[:], h[:], 0.0)
            s2 = pd.tile((128, nb, c.OUT), f16, tag="s2", name="s2")
            for q in range(nb):
                ps2 = psd.tile((128, c.OUT), f32, space="PSUM")
                nc.tensor.matmul(ps2[:], hr[:, q * 128:(q + 1) * 128],
                                 w2_sb[:], start=True, stop=True)
                nc.scalar.activation(s2[:, q, :], ps2[:], COPY)
            dst = bass.AP(tab2, (boff + b0) * 128 * 128,
                          [[128, 128], [128 * 128, nb], [1, c.OUT]])
            nc.sync.dma_start(dst, s2[:])

        # Phase F group: out rows = transpose(rs2 block) as f32
        def f_group(pf, psf, rsrc, boff, b0, nb):
            t = pf.tile((c.OUT, nb * 128), f16, tag="t", name="t")
            nc.sync.dma_start(
                t[:], rsrc.ap()[:, b0 * 128:(b0 + nb) * 128])
            of = pf.tile((128, nb, c.OUT), f32, tag="of", name="of")
            for q in range(nb):
                tp = psf.tile((128, c.OUT), f16, space="PSUM")
                nc.tensor.transpose(out=tp[:],
                                    in_=t[:, q * 128:(q + 1) * 128],
                                    identity=ident_sb[0:c.OUT, 0:c.OUT])
                nc.vector.tensor_copy(of[:, q, :], tp[:])
            gb = boff + b0
            rows = min(nb * 128, c.NPC - gb * 128)
            nfull = rows // 128
            if nfull > 0:
                dst = bass.AP(out, gb * 128 * c.OUT,
                              [[c.OUT, 128], [128 * c.OUT, nfull],
                               [1, c.OUT]])
                nc.sync.dma_start(dst, of[:, 0:nfull, :])
            rem = rows - nfull * 128
            if rem > 0:
                nc.sync.dma_start(
                    out.ap()[(gb + nfull) * 128:(gb + nfull) * 128 + rem, :],
                    of[0:rem, nfull, :])

        def scatter_layer(tab_ap, elem, estep, W, parts, gdt,
                          mid_cbs=(), job_lists=()):
            """Gather+scatter all dest cells in CELL_SEQ order. mid_cbs[s]
            fires shortly after split s completes; job_lists[s] are
            interleaved afterwards (one per cell)."""
            gq = [0]
            call_tile = {}
            seg_tile = {}
            if not rl2_loaded[0]:
                rl2_loaded[0] = True
                nc.sync.dma_start(rl2_sb[:], rowloc2[:])
            cb_at = {min(END_POS[s] + 10, len(CELL_SEQ) - 2): cb
                     for s, cb in enumerate(mid_cbs)}
            jobs_at = {END_POS[s] + 34: list(j)
                       for s, j in enumerate(job_lists)}
            live_jobs = []

            with tc.tile_pool(name="poh", bufs=10) as poh, \
                 tc.tile_pool(name="pix", bufs=2) as pix, \
                 tc.tile_pool(name="pg", bufs=22) as pg, \
                 tc.tile_pool(name="pem", bufs=5) as pem, \
                 tc.tile_pool(name="pso", bufs=2, space="PSUM") as pso:

                def ensure_call(qc):
                    if qc in call_tile:
                        return call_tile[qc]
                    ch = qc * GMAX
                    s = next(i for i in range(len(segb) - 1)
                             if ch < segb[i + 1])
                    if s not in seg_tile:
                        ch0, ch1 = segb[s], segb[s + 1]
                        st = pix.tile((128, (ch1 - ch0) * 8), i16,
                                      name="st", tag="st")
                        nc.sync.dma_start(
                            st[:], colidx.ap()[:, ch0 * 8:ch1 * 8])
                        seg_tile[s] = st
                    st = seg_tile[s]
                    sn = min(GMAX, nch - qc * GMAX)
                    off = (qc * GMAX - segb[s]) * 8
                    gt = pg.tile((128, sn, elem), gdt, name="gt", tag="gt")
                    _dma_gather_raw(
                        nc.gpsimd, gt[:], tab_ap, st[:, off:off + sn * 8],
                        sn * 128, sn * 128, elem, estep, gq[0] % 4)
                    gq[0] += 1
                    call_tile[qc] = (gt, sn)
                    return call_tile[qc]

                for pos, ce in enumerate(CELL_SEQ):
                    k = ce // CPK
                    loc = ce % CPK
                    sp = next(s for s in range(NSPL)
                              if loc < SPLITS[s + 1])
                    lh = loc - SPLITS[sp]
                    cb = int(qoff[ce])
                    m = int(m_cell[ce])
                    los = [int(blo[ce][i]) for i in range(NBLK)]
                    his = [int(bhi[ce][i]) for i in range(NBLK)]
                    cbase = [0]
                    for i in range(NBLK):
                        cbase.append(cbase[-1] + his[i] - los[i])
                    ncols = cbase[-1]
                    oh = poh.tile((128, ncols, 128), f16, name="oh",
                                  tag="oh")
                    for i in range(NBLK):
                        nco = his[i] - los[i]
                        ox = oh[:, cbase[i]:cbase[i + 1], :]
                        ov = bass.AP(ox.tensor, ox.offset,
                                     [ox.ap[0], [128, nco], [2, 64], [1, 2]])
                        ix = iota_sb[:, i * 128:(i + 1) * 128]
                        iv = bass.AP(ix.tensor, ix.offset,
                                     [ix.ap[0], [0, nco], [2, 64], [1, 2]])
                        j0 = cb + los[i]
                        rx = rl2_sb[:, 2 * j0:2 * j0 + 2 * nco]
                        rv = bass.AP(rx.tensor, rx.offset,
                                     [rx.ap[0], [2, nco], [0, 64], [1, 2]])
                        nc.vector.tensor_tensor(out=ov, in0=iv, in1=rv,
                                                op=iseq)
                    psU = pso.tile((W, 512), f32, space="PSUM",
                                   name="psU", tag="psU")
                    psV = pso.tile((W, 384), f32, space="PSUM",
                                   name="psV", tag="psV")
                    for i in range(NBLK):
                        pp = (psU[:, (i % 4) * 128:(i % 4) * 128 + 128]
                              if i < 4 else
                              psV[:, (i - 4) * 128:(i - 4) * 128 + 128])
                        for j in range(los[i], his[i]):
                            gi = cb + j
                            gt, sn = ensure_call(gi // GMAX)
                            sl = gt[:, gi % GMAX, 0:W]
                            nc.tensor.matmul(pp, sl,
                                             oh[:, cbase[i] + j - los[i], :],
                                             start=(j == los[i]),
                                             stop=(j == his[i] - 1))
                    emU = pem.tile((W, 512), f16, name="emU", tag="emU")
                    nc.scalar.activation(emU[:], psU[:], COPY)
                    emV = pem.tile((W, 384), f16, name="emV", tag="emV")
                    nc.scalar.activation(emV[:], psV[:], COPY)
                    i0 = lh * CELLR
                    nc.sync.dma_start(
                        parts[sp].ap()[k, :, i0:i0 + 512], emU[:])
                    nc.sync.dma_start(
                        parts[sp].ap()[k, :, i0 + 512:i0 + 896], emV[:])
                    if pos in cb_at:
                        cb_at.pop(pos)()
                    if pos in jobs_at:
                        live_jobs.extend(jobs_at.pop(pos))
                    if live_jobs:
                        live_jobs.pop(0)()
                for p in sorted(cb_at):
                    cb_at.pop(p)()
                for p in sorted(jobs_at):
                    live_jobs.extend(jobs_at.pop(p))
                while live_jobs:
                    live_jobs.pop(0)()

        # ---- Layer 1 ----
        pd = ctx.enter_context(tc.tile_pool(name="pd", bufs=3))
        psd = ctx.enter_context(tc.tile_pool(name="psd", bufs=2,
                                             space="PSUM"))
        sblk = [r // 128 for r in c.SROWS]          # blocks per split
        soff = [sum(sblk[:s]) for s in range(NSPL)]  # block offsets

        def rs_cb(ps, rs, s):
            def cb():
                nc.gpsimd.collective_compute(
                    "ReduceScatter", mybir.AluOpType.add,
                    replica_groups=groups,
                    ins=[ps[s].ap()], outs=[rs[s].ap()])
            return cb

        def grp_jobs(fn, pools, rs, s):
            return [(lambda b0=b0: fn(pools[0], pools[1], rs[s], soff[s],
                                      b0, min(4, sblk[s] - b0)))
                    for b0 in range(0, sblk[s], 4)]

        scatter_layer(
            tab1.ap()[:, 0:c.HID], c.HID, 2 * c.HID, c.HID, p1s, f8,
            mid_cbs=[rs_cb(p1s, r1s, s) for s in range(NSPL - 1)],
            job_lists=[grp_jobs(d_group, (pd, psd), r1s, s)
                       for s in range(NSPL - 1)])
        rs_cb(p1s, r1s, NSPL - 1)()
        for j in grp_jobs(d_group, (pd, psd), r1s, NSPL - 1):
            j()

        # ---- Layer 2 ----
        pf = ctx.enter_context(tc.tile_pool(name="pf", bufs=3))
        psf = ctx.enter_context(tc.tile_pool(name="psf", bufs=2,
                                             space="PSUM"))
        scatter_layer(
            tab2.ap()[:, 0:c.OUT], c.OUT, 128, c.OUT, p2s, f16,
            mid_cbs=[rs_cb(p2s, r2s, s) for s in range(NSPL - 1)],
            job_lists=[grp_jobs(f_group, (pf, psf), r2s, s)
                       for s in range(NSPL - 1)])
        rs_cb(p2s, r2s, NSPL - 1)()
        for j in grp_jobs(f_group, (pf, psf), r2s, NSPL - 1):
            j()

    nc.compile()
    return nc


def make_inputs(cfg, features, edge_index, W1, W2):
    c = cfg
    colidx, rowloc2, meta = prep_edges(cfg, edge_index)
    iota2d = np.broadcast_to(np.arange(CELLR, dtype=np.float16),
                             (128, CELLR)).copy()
    ident = np.eye(128, dtype=np.float16)
    w1 = np.ascontiguousarray(np.asarray(W1, np.float16))
    w2 = np.ascontiguousarray(np.asarray(W2, np.float16))
    in_maps = []
    for cc in range(c.NCORE):
        xc = np.asarray(features[cc * c.NPC:(cc + 1) * c.NPC], np.float32)
        xt = np.zeros((c.IN, c.NPP), np.float16)
        xt[:, :c.NPC] = xc.T.astype(np.float16)
        in_maps.append({
            "xT": np.ascontiguousarray(xt),
            "w1": w1, "w2": w2, "ident": ident, "iotaAB": iota2d,
            "colidx": np.ascontiguousarray(colidx[cc]),
            "rowloc2": np.ascontiguousarray(rowloc2[cc]),
        })
    return in_maps, meta


_LAST_NC = None


def kernel(features, edge_index, W1, W2):
    global _LAST_NC
    cfg = CFG
    in_maps, meta = make_inputs(cfg, features, edge_index, W1, W2)
    nc = build(cfg, meta)
    _LAST_NC = nc
    res = bass_utils.run_bass_kernel_spmd(
        nc, in_maps, core_ids=list(range(cfg.NCORE)))
    return np.concatenate(
        [res.results[cc]["out"] for cc in range(cfg.NCORE)], axis=0)
